# revision 5
# baseline (speedup 1.0000x reference)
"""Chamfer loss on 8 Trainium2 NeuronCores.

Sharding: data-parallel over the batch dim — core b handles batch element b
(one 4096x4096 distance problem per core), emits a single partial sum
S_b = sum_n min_m dist + sum_m min_n dist; the host combines the 8 scalars.

Per-core algorithm:
  d2[n,m] = ||x_n||^2 + ||y_m||^2 - 2 x_n.y_m  is produced by ONE K=11 fp16
  matmul per tile: the x.y term is computed in split precision
  (x ~ xh + xl, y ~ yh + yl in fp16; xy ~ xh*yh + xh*yl + xl*yh) which keeps
  the d2 error ~1e-6 while running at the PE's fast 1-cycle/row rate (true
  fp32 matmul is 4x slower).  ||y||^2 rides in the matmul as an fp16 hi/lo
  pair; ||x||^2 is added exactly (fp32) as the per-partition ACT bias during
  the PSUM->SBUF relu+fp16 conversion.

  Row minima (over m): fp16 tensor_tensor min tree on DVE (2x mode).
  Col minima (over n): fp16 running elementwise min into colacc, then a
  PE-transpose + reduce_min partition reduction at the end.
  sqrt+sum fused via ACT accum_out; partition sum via a ones-matmul.
"""

import os
import sys

import numpy as np

for _p in ("/opt/trn_rl_repo",):
    if _p not in sys.path and os.path.isdir(_p):
        sys.path.insert(0, _p)

B = 8          # batch (== number of cores)
N = 4096       # points per set
D = 3          # point dim
P = 128        # partitions
NT = N // P    # 32 n-tiles of 128 points
MCHUNK = 2048  # m processed per PSUM mega-tile (4 banks)
NJC = N // MCHUNK  # 2 chunks
MM_N = 512     # matmul moving free dim (one PSUM bank)
K = 11         # contraction: xh(3) xh(3) xl(3) 1 1


def build_nc(n=N, mchunk=MCHUNK):
    """Build the single-core Bass program (SPMD across 8 cores)."""
    import concourse.mybir as mybir
    import concourse.tile as tile
    from concourse import bacc
    from concourse.masks import make_identity

    dt = mybir.dt
    Alu = mybir.AluOpType
    Act = mybir.ActivationFunctionType
    AX = mybir.AxisListType

    nt = n // P
    njc = n // mchunk
    banks = mchunk // MM_N

    nc = bacc.Bacc("TRN2", target_bir_lowering=False, debug=False)

    x_dram = nc.dram_tensor("x", [n, D], dt.float32, kind="ExternalInput")
    y_dram = nc.dram_tensor("y", [n, D], dt.float32, kind="ExternalInput")
    out_dram = nc.dram_tensor("out", [1, 1], dt.float32, kind="ExternalOutput")

    with tile.TileContext(nc) as tc:
        with tc.tile_pool(name="singles", bufs=1) as singles:
            ident = singles.tile([P, P], dt.float16)
            make_identity(nc, ident)
            ones_f32 = singles.tile([P, 1], dt.float32)
            nc.gpsimd.memset(ones_f32, 1.0)

            x_nat = singles.tile([P, nt, D], dt.float32)
            y_nat = singles.tile([P, nt, D], dt.float32)
            # point index n = p*nt + t  (p outer) -> per-partition contiguous DMA
            nc.sync.dma_start(
                out=x_nat, in_=x_dram.ap().rearrange("(p t) d -> p t d", t=nt)
            )
            nc.sync.dma_start(
                out=y_nat, in_=y_dram.ap().rearrange("(p t) d -> p t d", t=nt)
            )

            natX = singles.tile([P, nt, K], dt.float16)
            natY = singles.tile([P, nt, K], dt.float16)
            x2cols = singles.tile([P, nt], dt.float32)
            y2_f32 = singles.tile([P, nt], dt.float32)
            sq_scr = singles.tile([P, nt, D], dt.float32)

            # --- X natural-layout aug: [xh xh xl 1 1] ---
            nc.gpsimd.memset(natX, 1.0)  # cols 9,10 stay 1
            nc.scalar.copy(out=natX[:, :, 0:3], in_=x_nat)          # xh = f16(x)
            nc.vector.tensor_copy(out=natX[:, :, 3:6], in_=natX[:, :, 0:3])
            nc.vector.tensor_tensor(                                 # xl = f16(x - xh)
                out=natX[:, :, 6:9], in0=x_nat, in1=natX[:, :, 0:3],
                op=Alu.subtract,
            )
            nc.vector.tensor_mul(sq_scr, x_nat, x_nat)
            nc.vector.tensor_reduce(
                out=x2cols, in_=sq_scr, axis=AX.X, op=Alu.add
            )

            # --- Y natural-layout aug: [-2yh -2yl -2yh y2h y2l] ---
            yh_tmp = singles.tile([P, nt, D], dt.float16)
            nc.scalar.copy(out=yh_tmp, in_=y_nat)                    # yh = f16(y)
            nc.scalar.mul(out=natY[:, :, 0:3], in_=yh_tmp, mul=-2.0)
            nc.vector.tensor_copy(out=natY[:, :, 6:9], in_=natY[:, :, 0:3])
            # -2*yl = (y * -2) - (-2yh)
            nc.vector.scalar_tensor_tensor(
                out=natY[:, :, 3:6], in0=y_nat, scalar=-2.0,
                in1=natY[:, :, 0:3], op0=Alu.mult, op1=Alu.subtract,
            )
            nc.vector.tensor_mul(sq_scr, y_nat, y_nat)
            nc.vector.tensor_reduce(
                out=y2_f32, in_=sq_scr, axis=AX.X, op=Alu.add
            )
            nc.scalar.copy(out=natY[:, :, 9:10], in_=y2_f32)         # y2h
            nc.vector.tensor_tensor(                                 # y2l
                out=natY[:, :, 10:11], in0=y2_f32, in1=natY[:, :, 9:10],
                op=Alu.subtract,
            )

            # --- transpose to K-major: Xaug/Yaug [K, n] fp16 ---
            Xaug = singles.tile([K, n], dt.float16)
            Yaug = singles.tile([K, n], dt.float16)
            with tc.tile_pool(name="psum_prep", bufs=2, space="PSUM") as pp:
                for (nat, aug) in ((natX, Xaug), (natY, Yaug)):
                    for g in range(nt // 4):
                        ps = pp.tile([K, 4 * P], dt.float16)
                        for q in range(4):
                            nc.tensor.transpose(
                                ps[:, q * P:(q + 1) * P], nat[:, g * 4 + q, :], ident
                            )
                        nc.scalar.copy(
                            out=aug[:, g * 4 * P:(g + 1) * 4 * P], in_=ps
                        )

            colacc = singles.tile([P, njc, mchunk], dt.float16)
            nc.gpsimd.memset(colacc, 60000.0)
            rowmins = singles.tile([P, nt], dt.float32)
            colmins = singles.tile([P, nt], dt.float32)

            # --- main loop ---
            with (
                tc.tile_pool(name="psum_main", bufs=2, space="PSUM") as pm,
                tc.tile_pool(name="hpool", bufs=4) as hp,
                tc.tile_pool(name="rpool", bufs=2) as rp,
            ):
                for i in range(nt):
                    h_prev = None
                    for jc in range(njc):
                        ps = pm.tile([P, mchunk], dt.float32)
                        for q in range(banks):
                            m0 = jc * mchunk + q * MM_N
                            nc.tensor.matmul(
                                ps[:, q * MM_N:(q + 1) * MM_N],
                                lhsT=Xaug[:, i * P:(i + 1) * P],
                                rhs=Yaug[:, m0:m0 + MM_N],
                                start=True, stop=True,
                            )
                        h = hp.tile([P, mchunk], dt.float16)
                        # h = f16(relu(d2)) ; adds exact fp32 ||x||^2 bias
                        nc.scalar.activation(
                            out=h, in_=ps, func=Act.Relu,
                            bias=x2cols[:, i:i + 1], scale=1.0,
                        )
                        nc.vector.tensor_tensor(
                            out=colacc[:, jc, :], in0=colacc[:, jc, :], in1=h,
                            op=Alu.min,
                        )
                        if jc == 0:
                            h_prev = h
                        else:
                            rowelem = rp.tile([P, mchunk], dt.float16)
                            nc.vector.tensor_tensor(
                                out=rowelem, in0=h_prev, in1=h, op=Alu.min
                            )
                            w = mchunk // 2
                            while w >= 64:
                                nc.vector.tensor_tensor(
                                    out=rowelem[:, 0:w], in0=rowelem[:, 0:w],
                                    in1=rowelem[:, w:2 * w], op=Alu.min,
                                )
                                w //= 2
                            nc.vector.tensor_reduce(
                                out=rowmins[:, i:i + 1], in_=rowelem[:, 0:64],
                                axis=AX.X, op=Alu.min,
                            )
                    if njc == 1:
                        # small-n fallback: tree directly on the single chunk
                        rowelem = rp.tile([P, mchunk], dt.float16)
                        nc.vector.tensor_copy(out=rowelem, in_=h_prev)
                        w = mchunk // 2
                        while w >= 64:
                            nc.vector.tensor_tensor(
                                out=rowelem[:, 0:w], in0=rowelem[:, 0:w],
                                in1=rowelem[:, w:2 * w], op=Alu.min,
                            )
                            w //= 2
                        nc.vector.tensor_reduce(
                            out=rowmins[:, i:i + 1], in_=rowelem[:, 0:64],
                            axis=AX.X, op=Alu.min,
                        )

            # --- column partition-reduction: PE transpose + reduce_min ---
            ngroups = (njc * mchunk) // (8 * P)  # groups of 8 [128,128] tiles
            with tc.tile_pool(name="psum_end", bufs=2, space="PSUM") as pe_pool:
                for g in range(ngroups):
                    pst = pe_pool.tile([P, 8, P], dt.float16)
                    for k in range(8):
                        base = g * 8 * P + k * P
                        jc, off = divmod(base, mchunk)
                        nc.tensor.transpose(
                            pst[:, k, :], colacc[:, jc, off:off + P], ident
                        )
                    nc.vector.tensor_reduce(
                        out=colmins[:, g * 8:(g + 1) * 8], in_=pst,
                        axis=AX.X, op=Alu.min,
                    )

                # --- sqrt, sums, partition sum ---
                sqs = singles.tile([P, nt], dt.float32)
                rowsum = singles.tile([P, 1], dt.float32)
                colsum = singles.tile([P, 1], dt.float32)
                nc.scalar.activation(
                    out=sqs, in_=rowmins, func=Act.Sqrt, accum_out=rowsum
                )
                sqs2 = singles.tile([P, nt], dt.float32)
                nc.scalar.activation(
                    out=sqs2, in_=colmins, func=Act.Sqrt, accum_out=colsum
                )
                total = singles.tile([P, 1], dt.float32)
                nc.vector.tensor_add(total, rowsum, colsum)
                ps_sum = pe_pool.tile([1, 1], dt.float32)
                nc.tensor.matmul(
                    ps_sum, lhsT=total, rhs=ones_f32, start=True, stop=True
                )
                res_sb = singles.tile([1, 1], dt.float32)
                nc.scalar.copy(out=res_sb, in_=ps_sum)
                nc.sync.dma_start(out=out_dram.ap(), in_=res_sb)

    nc.compile()
    return nc


_NC_CACHE = {}


def _get_nc():
    if "nc" not in _NC_CACHE:
        _NC_CACHE["nc"] = build_nc()
    return _NC_CACHE["nc"]


def kernel(set1, set2):
    from concourse import bass_utils

    set1 = np.asarray(set1, dtype=np.float32)
    set2 = np.asarray(set2, dtype=np.float32)
    assert set1.shape == (B, N, D) and set2.shape == (B, N, D)

    nc = _get_nc()
    in_maps = [
        {"x": np.ascontiguousarray(set1[b]), "y": np.ascontiguousarray(set2[b])}
        for b in range(B)
    ]
    res = bass_utils.run_bass_kernel_spmd(nc, in_maps, core_ids=list(range(B)))
    parts = np.array(
        [np.asarray(res.results[b]["out"]).reshape(()) for b in range(B)],
        dtype=np.float64,
    )
    total = parts.sum() / (B * N) / N
    return np.float32(total)


# revision 19
# speedup vs baseline: 1378.9839x; 1378.9839x over previous
"""Chamfer loss on 8 Trainium2 NeuronCores.

Sharding: data-parallel over the batch dim — core b handles batch element b
(one 4096x4096 distance problem per core), emits a single partial sum
S_b = sum_n min_m dist + sum_m min_n dist; the host combines the 8 scalars.

Per-core algorithm:
  d2[n,m] = ||x_n||^2 + ||y_m||^2 - 2 x_n.y_m  is produced by ONE K=11 fp16
  matmul per tile: the x.y term is computed in split precision
  (x ~ xh + xl, y ~ yh + yl in fp16; xy ~ xh*yh + xh*yl + xl*yh) which keeps
  the d2 error ~1e-6 while running at the PE's fast 1-cycle/row rate (true
  fp32 matmul is 4x slower).  ||y||^2 rides in the matmul as an fp16 hi/lo
  pair; ||x||^2 is added exactly (fp32) as the per-partition ACT bias during
  the PSUM->SBUF relu+fp16 conversion.

  Row minima (over m): fp16 tensor_tensor min tree on DVE (2x mode).
  Col minima (over n): fp16 running elementwise min into colacc, then a
  PE-transpose + reduce_min partition reduction at the end.
  sqrt+sum fused via ACT accum_out; partition sum via a ones-matmul.

build_nc(reps=R) unrolls the whole per-core computation R times;
build_nc(loop_reps=R) wraps it in a hardware For_i loop instead (same NEFF
size for any R — used by the timing harness).
"""

import os
import sys

import numpy as np

for _p in ("/opt/trn_rl_repo",):
    if _p not in sys.path and os.path.isdir(_p):
        sys.path.insert(0, _p)

B = 8          # batch (== number of cores)
N = 4096       # points per set
D = 3          # point dim
P = 128        # partitions
MCHUNK = 2048  # m processed per PSUM mega-tile (4 banks)
MM_N = 512     # matmul moving free dim (one PSUM bank)
K = 11         # contraction: xh(3) xh(3) xl(3) 1 1


def build_nc(n=N, mchunk=MCHUNK, reps=1, loop_reps=0, skip=()):
    """Build the single-core Bass program (SPMD across 8 cores).

    skip: dev-only ablation switches for timing breakdowns
          (subset of {"mm", "act", "col", "row"}).  Skipping breaks
          numerics; only the full build is used for real runs.
    """
    skip = frozenset(skip)
    import concourse.mybir as mybir
    import concourse.tile as tile
    from concourse import bacc
    from concourse.masks import make_identity

    dt = mybir.dt
    Alu = mybir.AluOpType
    Act = mybir.ActivationFunctionType
    AX = mybir.AxisListType

    nt = n // P
    njc = n // mchunk
    banks = mchunk // MM_N

    nc = bacc.Bacc("TRN2", target_bir_lowering=False, debug=False)

    x_dram = nc.dram_tensor("x", [n, D], dt.float32, kind="ExternalInput")
    y_dram = nc.dram_tensor("y", [n, D], dt.float32, kind="ExternalInput")
    out_dram = nc.dram_tensor("out", [1, 1], dt.float32, kind="ExternalOutput")

    with tile.TileContext(nc) as tc:
        with tc.tile_pool(name="singles", bufs=1) as singles:
            ident = singles.tile([P, P], dt.float16)
            make_identity(nc, ident)
            ones_f32 = singles.tile([P, 1], dt.float32)
            nc.gpsimd.memset(ones_f32, 1.0)

            x_nat = singles.tile([P, nt, D], dt.float32)
            y_nat = singles.tile([P, nt, D], dt.float32)
            natX = singles.tile([P, nt, K], dt.float16)
            natY = singles.tile([P, nt, K], dt.float16)
            x2cols = singles.tile([P, nt], dt.float32)
            y2_f32 = singles.tile([P, nt], dt.float32)
            sq_scr = singles.tile([P, nt, D], dt.float32)
            yh_tmp = singles.tile([P, nt, D], dt.float16)
            Xaug = singles.tile([K, n], dt.float16)
            Yaug = singles.tile([K, n], dt.float16)
            colacc = singles.tile([P, njc, mchunk], dt.float16)
            rowmins = singles.tile([P, nt], dt.float32)
            colmins = singles.tile([P, nt], dt.float32)
            sqs = singles.tile([P, nt], dt.float32)
            sqs2 = singles.tile([P, nt], dt.float32)
            rowsum = singles.tile([P, 1], dt.float32)
            colsum = singles.tile([P, 1], dt.float32)
            total = singles.tile([P, 1], dt.float32)
            res_sb = singles.tile([1, 1], dt.float32)

            def emit_iteration(tag):
                # ---------- load + natural-layout aug ----------
                # point index n = p*nt + t (p outer) -> contiguous DMA
                nc.sync.dma_start(
                    out=x_nat, in_=x_dram.ap().rearrange("(p t) d -> p t d", t=nt)
                )
                nc.sync.dma_start(
                    out=y_nat, in_=y_dram.ap().rearrange("(p t) d -> p t d", t=nt)
                )

                # X: [xh xh xl 1 1]
                nc.gpsimd.memset(natX, 1.0)  # cols 9,10 stay 1
                nc.scalar.copy(out=natX[:, :, 0:3], in_=x_nat)     # xh = f16(x)
                nc.vector.tensor_copy(out=natX[:, :, 3:6], in_=natX[:, :, 0:3])
                nc.vector.tensor_tensor(                            # xl = f16(x-xh)
                    out=natX[:, :, 6:9], in0=x_nat, in1=natX[:, :, 0:3],
                    op=Alu.subtract,
                )
                nc.vector.tensor_mul(sq_scr, x_nat, x_nat)
                nc.vector.tensor_reduce(out=x2cols, in_=sq_scr, axis=AX.X,
                                        op=Alu.add)

                # Y: [-2yh -2yl -2yh y2h y2l]
                nc.scalar.copy(out=yh_tmp, in_=y_nat)               # yh = f16(y)
                nc.scalar.mul(out=natY[:, :, 0:3], in_=yh_tmp, mul=-2.0)
                nc.vector.tensor_copy(out=natY[:, :, 6:9], in_=natY[:, :, 0:3])
                nc.vector.scalar_tensor_tensor(                     # -2yl
                    out=natY[:, :, 3:6], in0=y_nat, scalar=-2.0,
                    in1=natY[:, :, 0:3], op0=Alu.mult, op1=Alu.subtract,
                )
                nc.vector.tensor_mul(sq_scr, y_nat, y_nat)
                nc.vector.tensor_reduce(out=y2_f32, in_=sq_scr, axis=AX.X,
                                        op=Alu.add)
                nc.scalar.copy(out=natY[:, :, 9:10], in_=y2_f32)    # y2h
                nc.vector.tensor_tensor(                            # y2l
                    out=natY[:, :, 10:11], in0=y2_f32, in1=natY[:, :, 9:10],
                    op=Alu.subtract,
                )

                # ---------- transpose to K-major ----------
                with tc.tile_pool(name=f"pp{tag}", bufs=2, space="PSUM") as pp:
                    for (nat, aug) in ((natX, Xaug), (natY, Yaug)):
                        for g in range(nt // 4):
                            ps = pp.tile([K, 4 * P], dt.float16, tag="tp")
                            for q in range(4):
                                nc.tensor.transpose(
                                    ps[:, q * P:(q + 1) * P],
                                    nat[:, g * 4 + q, :], ident,
                                )
                            nc.vector.tensor_copy(
                                out=aug[:, g * 4 * P:(g + 1) * 4 * P], in_=ps
                            )

                nc.gpsimd.memset(colacc, 60000.0)
                if skip:
                    nc.gpsimd.memset(rowmins, 1.0)
                    nc.gpsimd.memset(colmins, 1.0)

                # ---------- main loop ----------
                with (
                    tc.tile_pool(name=f"pm{tag}", bufs=2, space="PSUM") as pm,
                    tc.tile_pool(name=f"hp{tag}", bufs=4) as hp,
                    tc.tile_pool(name=f"rp{tag}", bufs=2) as rp,
                ):
                    for i in range(nt):
                        h_prev = None
                        for jc in range(njc):
                            ps = pm.tile([P, mchunk], dt.float32, tag="d2")
                            if "mm" not in skip:
                                for q in range(banks):
                                    m0 = jc * mchunk + q * MM_N
                                    nc.tensor.matmul(
                                        ps[:, q * MM_N:(q + 1) * MM_N],
                                        lhsT=Xaug[:, i * P:(i + 1) * P],
                                        rhs=Yaug[:, m0:m0 + MM_N],
                                        start=True, stop=True,
                                    )
                            if "act" in skip:
                                continue
                            h = hp.tile([P, mchunk], dt.float16, tag="h")
                            # h = f16(relu(d2 + ||x||^2))
                            nc.scalar.activation(
                                out=h, in_=ps, func=Act.Relu,
                                bias=x2cols[:, i:i + 1], scale=1.0,
                            )
                            if "col" not in skip:
                                nc.vector.tensor_tensor(
                                    out=colacc[:, jc, :], in0=colacc[:, jc, :],
                                    in1=h, op=Alu.min,
                                )
                            if "row" in skip:
                                continue
                            if jc == 0:
                                h_prev = h
                            else:
                                rowelem = rp.tile([P, mchunk], dt.float16,
                                                  tag="re")
                                nc.vector.tensor_tensor(
                                    out=rowelem, in0=h_prev, in1=h, op=Alu.min
                                )
                                w = mchunk // 2
                                while w >= 64:
                                    nc.vector.tensor_tensor(
                                        out=rowelem[:, 0:w], in0=rowelem[:, 0:w],
                                        in1=rowelem[:, w:2 * w], op=Alu.min,
                                    )
                                    w //= 2
                                nc.vector.tensor_reduce(
                                    out=rowmins[:, i:i + 1],
                                    in_=rowelem[:, 0:64], axis=AX.X, op=Alu.min,
                                )
                        if njc == 1 and "row" not in skip and "act" not in skip:
                            rowelem = rp.tile([P, mchunk], dt.float16, tag="re")
                            nc.vector.tensor_copy(out=rowelem, in_=h_prev)
                            w = mchunk // 2
                            while w >= 64:
                                nc.vector.tensor_tensor(
                                    out=rowelem[:, 0:w], in0=rowelem[:, 0:w],
                                    in1=rowelem[:, w:2 * w], op=Alu.min,
                                )
                                w //= 2
                            nc.vector.tensor_reduce(
                                out=rowmins[:, i:i + 1], in_=rowelem[:, 0:64],
                                axis=AX.X, op=Alu.min,
                            )

                # ---------- column partition-reduction ----------
                ngroups = 0 if ("col" in skip or "act" in skip) \
                    else (njc * mchunk) // (8 * P)
                with tc.tile_pool(name=f"pe{tag}", bufs=2, space="PSUM") as pep:
                    for g in range(ngroups):
                        pst = pep.tile([P, 8, P], dt.float16, tag="ct")
                        for k in range(8):
                            base = g * 8 * P + k * P
                            jc, off = divmod(base, mchunk)
                            nc.tensor.transpose(
                                pst[:, k, :], colacc[:, jc, off:off + P], ident
                            )
                        nc.vector.tensor_reduce(
                            out=colmins[:, g * 8:(g + 1) * 8], in_=pst,
                            axis=AX.X, op=Alu.min,
                        )

                    # ---------- sqrt, sums, partition sum ----------
                    nc.scalar.activation(
                        out=sqs, in_=rowmins, func=Act.Sqrt, accum_out=rowsum
                    )
                    nc.scalar.activation(
                        out=sqs2, in_=colmins, func=Act.Sqrt, accum_out=colsum
                    )
                    nc.vector.tensor_add(total, rowsum, colsum)
                    ps_sum = pep.tile([1, 1], dt.float32, tag="pssum")
                    nc.tensor.matmul(
                        ps_sum, lhsT=total, rhs=ones_f32, start=True, stop=True
                    )
                    nc.scalar.copy(out=res_sb, in_=ps_sum)
                    nc.sync.dma_start(out=out_dram.ap(), in_=res_sb)

            if loop_reps:
                with tc.For_i(0, loop_reps, 1):
                    emit_iteration("L")
            else:
                for rep in range(reps):
                    emit_iteration(str(rep))
                    if reps > 1:
                        # serialize unrolled reps for standalone-latency timing
                        tc.strict_bb_all_engine_barrier()

    nc.compile()
    return nc


_NC_CACHE = {}


def _get_nc():
    if "nc" not in _NC_CACHE:
        _NC_CACHE["nc"] = build_nc()
    return _NC_CACHE["nc"]


def kernel(set1, set2):
    from concourse import bass_utils

    set1 = np.asarray(set1, dtype=np.float32)
    set2 = np.asarray(set2, dtype=np.float32)
    assert set1.shape == (B, N, D) and set2.shape == (B, N, D)

    nc = _get_nc()
    in_maps = [
        {"x": np.ascontiguousarray(set1[b]), "y": np.ascontiguousarray(set2[b])}
        for b in range(B)
    ]
    res = bass_utils.run_bass_kernel_spmd(nc, in_maps, core_ids=list(range(B)))
    parts = np.array(
        [np.asarray(res.results[b]["out"]).reshape(()) for b in range(B)],
        dtype=np.float64,
    )
    total = parts.sum() / (B * N) / N
    return np.float32(total)
